# revision 4
# baseline (speedup 1.0000x reference)
"""Trainium2 Bass kernel for nn_RadialPredictionLayer (retrieval_knn).

Computes out[n, c] = -sqrt(max(||x_n||^2 + ||p_c||^2 - 2 * x_n . p_c, 0))
for x [32768, 1024] fp32 and prototypes [1024, 1024] fp32.

The layer's prototypes are a fixed (non-trainable) identity matrix, so the
device kernel specializes on that constant (verified at runtime):
    cross = x @ I^T = x,  ||p_c||^2 = 1
    out[n, c] = -sqrt(1 + ||x_n||^2 - 2 * x[n, c])
which is a pure memory-bound elementwise + row-reduction kernel (no GEMM).
Sharding: data-parallel on the batch axis across 8 NeuronCores; each core
processes a [4096, 1024] row block. If prototypes is ever not the identity,
a host-side exact fallback implements the general formula.
"""

import numpy as np

N_CORES = 8
N_ROWS = 32768
D = 1024
ROWS_PER_CORE = N_ROWS // N_CORES  # 4096
T = 4  # rows per partition per super-tile
SUP = ROWS_PER_CORE // (128 * T)  # super-tiles per core

_cache = {}


def _build_program(rows=ROWS_PER_CORE, debug=False):
    import concourse.bacc as bacc
    import concourse.mybir as mybir
    import concourse.tile as tile

    f32 = mybir.dt.float32
    nc = bacc.Bacc("TRN2", target_bir_lowering=False, debug=debug)
    x = nc.dram_tensor("x", [rows, D], f32, kind="ExternalInput").ap()
    out = nc.dram_tensor("out", [rows, D], f32, kind="ExternalOutput").ap()

    xv = x.rearrange("(s p t) d -> s p (t d)", p=128, t=T)
    ov = out.rearrange("(s p t) d -> s p (t d)", p=128, t=T)

    with tile.TileContext(nc) as tc:
        with (
            tc.tile_pool(name="xt", bufs=4) as xpool,
            tc.tile_pool(name="sc", bufs=2) as scpool,
            tc.tile_pool(name="b", bufs=4) as bpool,
        ):
            for s in range(rows // (128 * T)):
                xt = xpool.tile([128, T * D], f32)
                nc.sync.dma_start(out=xt[:], in_=xv[s])
                b = bpool.tile([128, T], f32)
                sq = scpool.tile([128, D], f32)
                for t in range(T):
                    blk = xt[:, t * D : (t + 1) * D]
                    # sq = x*x (discarded); b[:, t] = sum(x*x) per row
                    nc.vector.scalar_tensor_tensor(
                        out=sq[:],
                        in0=blk,
                        scalar=1.0,
                        in1=blk,
                        op0=mybir.AluOpType.mult,
                        op1=mybir.AluOpType.mult,
                        accum_out=b[:, t : t + 1],
                    )
                # b = 1 + ||x_row||^2
                nc.vector.tensor_scalar_add(out=b[:], in0=b[:], scalar1=1.0)
                for t in range(T):
                    blk = xt[:, t * D : (t + 1) * D]
                    # blk = sqrt(-2*x + (1 + ||x_row||^2))   (in place)
                    nc.scalar.activation(
                        out=blk,
                        in_=blk,
                        func=mybir.ActivationFunctionType.Sqrt,
                        bias=b[:, t : t + 1],
                        scale=-2.0,
                    )
                # negate the whole super-tile in one op
                nc.vector.tensor_scalar_mul(out=xt[:], in0=xt[:], scalar1=-1.0)
                nc.sync.dma_start(out=ov[s], in_=xt[:])
    nc.finalize()
    return nc


def _run_device(x: np.ndarray, trace: bool = False):
    from concourse import bass_utils

    if "nc" not in _cache:
        _cache["nc"] = _build_program()
    nc = _cache["nc"]
    shards = [
        np.ascontiguousarray(x[i * ROWS_PER_CORE : (i + 1) * ROWS_PER_CORE])
        for i in range(N_CORES)
    ]
    res = bass_utils.run_bass_kernel_spmd(
        nc,
        [{"x": s} for s in shards],
        core_ids=list(range(N_CORES)),
        trace=trace,
    )
    out = np.concatenate([r["out"] for r in res.results], axis=0)
    return out, res


def _fallback(x: np.ndarray, prototypes: np.ndarray) -> np.ndarray:
    x = x.astype(np.float32, copy=False)
    p = prototypes.astype(np.float32, copy=False)
    x_sq = np.sum(x * x, axis=1, keepdims=True)
    p_sq = np.sum(p * p, axis=1)
    cross = x @ p.T
    d2 = np.maximum(x_sq + p_sq[None, :] - 2.0 * cross, 0.0)
    return (-np.sqrt(d2)).astype(np.float32)


def _is_identity(p: np.ndarray) -> bool:
    if p.shape != (D, D):
        return False
    if "eye" not in _cache:
        _cache["eye"] = np.eye(D, dtype=np.float32)
    return np.array_equal(np.asarray(p, dtype=np.float32), _cache["eye"])


def kernel(x: np.ndarray, prototypes: np.ndarray) -> np.ndarray:
    x = np.asarray(x)
    prototypes = np.asarray(prototypes)
    if (
        x.shape == (N_ROWS, D)
        and x.dtype == np.float32
        and _is_identity(prototypes)
    ):
        out, _ = _run_device(x)
        return out
    return _fallback(x, prototypes)
